# revision 12
# baseline (speedup 1.0000x reference)
"""BigBird attention (S=4096, D=1024, H=16, window=256, 1 global, 32 random)
on 8 TRN2 NeuronCores, head-sharded (2 heads / core).

v2: qb-pair t-major attention. Per core c (heads 2c..2c+1, features c*128..):
  - phase B: projections kT/vT (+qT for qb0/qb1), sc-chunk-paired matmuls to
    amortize stationary reloads
  - phase C: 4 query-block pairs (1024 queries each), t-major over 32 ktiles:
    scores (row-tiled 2-head concurrent, stationary kept resident across the
    pair), exp on ScalarE (the 1 elem/cycle/lane wall), merged mask multiply
    via stride-0 broadcast AP, attnV with ones-column denominators (M=65)
    with stationary reused across the pair
  - normalization + out-proj in a compact pair tail; host sums 8 partials
    and adds the constant bv @ Wo.T + bo term.
"""

import os
from contextlib import ExitStack

import numpy as np
import ml_dtypes

import concourse.bass as bass
import concourse.bacc as bacc
import concourse.tile as tile
from concourse import mybir
import concourse.bass_utils as bass_utils

bass_utils.upload_artifacts = lambda tmpdir: "local"

F32 = mybir.dt.float32
BF16 = mybir.dt.bfloat16
AFT = mybir.ActivationFunctionType

S = 4096          # sequence length
D = 1024          # d_model
F = 128           # features per core (2 heads x 64)
DH = 64           # head dim
NCORES = 8
KF = 8            # contraction chunks of 128 over D
SC = 8            # seq chunks of 512 (projections)
KT = 32           # key tiles of 128
QB = 8            # query blocks of 512
VW = 2 * (DH + 1)  # per-ktile v_aug width (65 per head)

LAST_EXEC_TIME_NS = None
LAST_RESULT = None

_NC_CACHE = None


def _build_nc():
    nc = bacc.Bacc("TRN2", target_bir_lowering=False, debug=False)

    xT = nc.declare_dram_parameter("xT", [D, S], BF16, isOutput=False)
    WqT = nc.declare_dram_parameter("WqT", [D, F], BF16, isOutput=False)
    WkT = nc.declare_dram_parameter("WkT", [D, F], BF16, isOutput=False)
    WvT = nc.declare_dram_parameter("WvT", [D, F], BF16, isOutput=False)
    bqv = nc.declare_dram_parameter("bqv", [F, 1], F32, isOutput=False)
    bkv = nc.declare_dram_parameter("bkv", [F, 1], F32, isOutput=False)
    WoT = nc.declare_dram_parameter("WoT", [F, D], BF16, isOutput=False)
    maskT = nc.declare_dram_parameter("maskT", [S, S], BF16, isOutput=False)
    ident = nc.declare_dram_parameter("ident", [128, 128], BF16, isOutput=False)
    out = nc.declare_dram_parameter("out", [S, D], BF16, isOutput=True)

    with tile.TileContext(nc) as tc:
        with ExitStack() as ctx:
            # ---- persistent sbuf ----
            wpool = ctx.enter_context(tc.tile_pool(name="w", bufs=1))
            wq = wpool.tile([128, D], BF16, tag="wq")
            wk = wpool.tile([128, D], BF16, tag="wk")
            wv = wpool.tile([128, D], BF16, tag="wv")
            for kf in range(KF):
                sl = slice(kf * 128, (kf + 1) * 128)
                nc.sync.dma_start(wq[:, sl], WqT[sl, :])
                nc.sync.dma_start(wk[:, sl], WkT[sl, :])
                nc.sync.dma_start(wv[:, sl], WvT[sl, :])
            wo = wpool.tile([128, D], BF16, tag="wo")
            nc.sync.dma_start(wo[:], WoT[:, :])
            bq_sb = wpool.tile([128, 1], F32, tag="bq")
            bk_sb = wpool.tile([128, 1], F32, tag="bk")
            nc.sync.dma_start(bq_sb[:], bqv[:, :])
            nc.sync.dma_start(bk_sb[:], bkv[:, :])
            id_sb = wpool.tile([128, 128], BF16, tag="id")
            nc.sync.dma_start(id_sb[:], ident[:, :])

            qTs = [wpool.tile([128, 512], BF16, tag=f"qT{i}", name=f"qT{i}")
                   for i in range(QB)]
            kTs = [wpool.tile([128, 512], BF16, tag=f"kT{i}", name=f"kT{i}")
                   for i in range(SC)]
            vs = [wpool.tile([128, VW], BF16, tag=f"v{t}", name=f"v{t}")
                  for t in range(KT)]
            for t in range(KT):
                nc.vector.memset(vs[t][:], 1.0)  # ones cols survive at 64/129

            def kt_ap(h, t):  # [64, 128] head h, key tile t
                return kTs[t // 4][h * DH:(h + 1) * DH,
                                   (t % 4) * 128:(t % 4 + 1) * 128]

            def qt_ap(h, qb):  # [64, 512] head h, query block qb
                return qTs[qb][h * DH:(h + 1) * DH, :]

            xpool = ctx.enter_context(tc.tile_pool(name="xs", bufs=24))

            # ---- phase B: projections, sc-pair stationary reuse ----
            with (
                tc.tile_pool(name="vt", bufs=2) as vtpool,
                tc.tile_pool(name="prps", bufs=4, space="PSUM") as prpool,
                tc.tile_pool(name="qps", bufs=2, space="PSUM") as qppool,
                tc.tile_pool(name="tpps", bufs=2, space="PSUM") as tppool,
            ):
                xts = {}

                def load_x(sc):
                    for kf in range(KF):
                        ksl = slice(kf * 128, (kf + 1) * 128)
                        ssl = slice(sc * 512, (sc + 1) * 512)
                        t_ = xpool.tile([128, 512], BF16, tag="xt",
                                        name=f"xt_{sc}_{kf}")
                        nc.gpsimd.dma_start(t_[:], xT[ksl, ssl])
                        xts[(sc, kf)] = t_

                for sc in range(2):
                    load_x(sc)

                for scp in range(4):
                    scA, scB = 2 * scp, 2 * scp + 1
                    if scA + 2 < SC:
                        load_x(scA + 2)
                    if scB + 2 < SC:
                        load_x(scB + 2)
                    pskA = prpool.tile([128, 512], F32, tag="pr",
                                        name=f"pskA_{scp}")
                    psvA = prpool.tile([128, 512], F32, tag="pr",
                                        name=f"psvA_{scp}")
                    pskB = prpool.tile([128, 512], F32, tag="pr",
                                        name=f"pskB_{scp}")
                    psvB = prpool.tile([128, 512], F32, tag="pr",
                                        name=f"psvB_{scp}")
                    for kf in range(KF):
                        ksl = slice(kf * 128, (kf + 1) * 128)
                        st, sp = kf == 0, kf == KF - 1
                        xA, xB = xts[(scA, kf)][:], xts[(scB, kf)][:]
                        nc.tensor.matmul(pskA[:], wk[:, ksl], xA,
                                         start=st, stop=sp)
                        nc.tensor.matmul(pskB[:], wk[:, ksl], xB,
                                         start=st, stop=sp)
                        nc.tensor.matmul(psvA[:], wv[:, ksl], xA,
                                         start=st, stop=sp)
                        nc.tensor.matmul(psvB[:], wv[:, ksl], xB,
                                         start=st, stop=sp)
                    for sc, psk, psv in ((scA, pskA, psvA), (scB, pskB, psvB)):
                        nc.scalar.activation(kTs[sc][:], psk[:], AFT.Identity,
                                             bias=bk_sb[:], scale=1.0)
                        vt = vtpool.tile([128, 512], BF16)
                        nc.scalar.activation(vt[:], psv[:], AFT.Identity)
                        for j in range(4):
                            t_g = sc * 4 + j
                            pst = tppool.tile([128, 128], BF16, tag="pst")
                            nc.tensor.transpose(pst[:],
                                                vt[:, j * 128:(j + 1) * 128],
                                                id_sb[:])
                            dst = vs[t_g][:].rearrange("p (h x) -> p h x",
                                                       h=2)[:, :, 0:DH]
                            src = pst[:].rearrange("p (h x) -> p h x", h=2)
                            nc.vector.tensor_copy(dst, src)

                    if scp == 0:
                        # q projections for qb0/qb1 while their x tiles live
                        for qb in (0, 1):
                            psq = qppool.tile([128, 512], F32, tag="psq",
                                              name=f"psqB_{qb}")
                            for kf in range(KF):
                                ksl = slice(kf * 128, (kf + 1) * 128)
                                nc.tensor.matmul(psq[:], wq[:, ksl],
                                                 xts[(qb, kf)][:],
                                                 start=(kf == 0),
                                                 stop=(kf == KF - 1))
                            nc.scalar.activation(qTs[qb][:], psq[:],
                                                 AFT.Identity,
                                                 bias=bq_sb[:], scale=1.0)


            # ---- phase C: attention, single-qb t-loops ----
            # PSUM: spool 2x[128,1024] (4 banks) + opool 3x[*,512] (3) +
            # popool 1 (dedicated out-proj bank -> no ps_s ring contention)
            LAG = 2
            with (
                tc.tile_pool(name="mask", bufs=10) as mpool,
                tc.tile_pool(name="attn", bufs=10) as atpool,
                tc.tile_pool(name="an", bufs=2) as anpool,
                tc.tile_pool(name="dn", bufs=8) as dpool,
                tc.tile_pool(name="rc", bufs=4) as rpool,
                tc.tile_pool(name="og", bufs=3) as ogpool,
                tc.tile_pool(name="sps", bufs=2, space="PSUM") as spool,
                tc.tile_pool(name="ops", bufs=3, space="PSUM") as opool,
                tc.tile_pool(name="pps", bufs=1, space="PSUM") as popool,
            ):
                def make_ops(qb, an):
                    # out-proj pieces, scattered into the next qb's t-loop;
                    # po has a dedicated bank so scores never wait on it
                    def op_piece(stt):
                        def go():
                            for oc in range(2):
                                po = popool.tile([128, 512], F32, tag="po",
                                                 name=f"po_{qb}_{stt}_{oc}")
                                nc.tensor.matmul(
                                    po[:],
                                    an[:, stt * 128:(stt + 1) * 128],
                                    wo[:, oc * 512:(oc + 1) * 512],
                                    start=True, stop=True)
                                og = ogpool.tile([128, 512], BF16, tag="og",
                                                 name=f"og_{qb}_{stt}_{oc}")
                                nc.vector.tensor_copy(og[:], po[:])
                                r0 = qb * 512 + stt * 128
                                nc.sync.dma_start(
                                    out[r0:r0 + 128,
                                        oc * 512:(oc + 1) * 512], og[:])
                        return go

                    return {2: op_piece(0), 6: op_piece(1),
                            10: op_piece(2), 14: op_piece(3)}

                def qb_tail(qb, ps_o, an):
                    # normalize at qb end
                    for h in range(2):
                        po_ = ps_o[h]
                        dn = dpool.tile([1, 512], F32, tag="dn",
                                        name=f"dn_{qb}_{h}")
                        nc.vector.tensor_copy(dn[:], po_[DH:DH + 1, :])
                        rc1 = dpool.tile([1, 512], F32, tag="rc1",
                                         name=f"rc1_{qb}_{h}")
                        nc.vector.reciprocal_approx_fast(rc1[:], dn[:])
                        rcb = rpool.tile([DH, 512], F32, tag="rc",
                                         name=f"rc_{qb}_{h}")
                        nc.gpsimd.partition_broadcast(rcb[:], rc1[:])
                        nc.vector.tensor_mul(an[h * DH:(h + 1) * DH, :],
                                             po_[0:DH, :], rcb[:])
                    # q projection for qb+2 (x prefetched at loop start);
                    # psq borrows an opool slot the eps above freed
                    if qb + 2 < QB:
                        qb_n = qb + 2
                        psq = opool.tile([128, 512], F32, tag="ps_o",
                                         name=f"psq_{qb_n}")
                        for kf in range(KF):
                            ksl2 = slice(kf * 128, (kf + 1) * 128)
                            nc.tensor.matmul(psq[:], wq[:, ksl2],
                                             xqs[(qb_n, kf)][:],
                                             start=(kf == 0),
                                             stop=(kf == KF - 1))
                        nc.scalar.activation(qTs[qb_n][:], psq[:],
                                             AFT.Identity,
                                             bias=bq_sb[:], scale=1.0)

                ops = {}
                xqs = {}
                for qb in range(QB):
                    qsl = slice(qb * 512, (qb + 1) * 512)
                    ps_o = {}
                    ats = {}

                    def emit_attnv(t, first):
                        at = ats.pop(t)
                        for h in range(2):
                            vst = vs[t][:, h * (DH + 1):(h + 1) * (DH + 1)]
                            nc.tensor.matmul(
                                ps_o[h][:], vst,
                                at[:, h * 512:(h + 1) * 512],
                                start=first, stop=(t == KT - 1))

                    for t in range(KT):
                        tsl = slice(t * 128, (t + 1) * 128)
                        msk = mpool.tile([128, 512], BF16)
                        nc.sync.dma_start(msk[:], maskT[tsl, qsl])
                        if qb + 2 < QB and t == 0:
                            # prefetch x for qb+2's q projection
                            qb_n = qb + 2
                            ssl2 = slice(qb_n * 512, (qb_n + 1) * 512)
                            for kf in range(KF):
                                ksl2 = slice(kf * 128, (kf + 1) * 128)
                                xt2 = xpool.tile([128, 512], BF16, tag="xt",
                                                 name=f"xt2_{qb_n}_{kf}")
                                nc.gpsimd.dma_start(xt2[:], xT[ksl2, ssl2])
                                xqs[(qb_n, kf)] = xt2
                        ps_s = spool.tile([128, 1024], F32, tag="ps_s",
                                          name=f"ps_s_{qb}_{t}")
                        for h in range(2):
                            nc.tensor.matmul(
                                ps_s[:, h * 512:(h + 1) * 512],
                                kt_ap(h, t), qt_ap(h, qb),
                                start=True, stop=True,
                                tile_position=(h * DH, 0))
                        at = atpool.tile([128, 1024], BF16, tag="at",
                                         name=f"at_{qb}_{t}")
                        nc.scalar.activation(at[:], ps_s[:], AFT.Exp)
                        # merged mask multiply via stride-0 broadcast
                        mB = msk[:].unsqueeze(1).broadcast_to([128, 2, 512])
                        at3 = at[:].rearrange("p (o x) -> p o x", o=2)
                        nc.vector.tensor_mul(at3, at3, mB)
                        ats[t] = at
                        if t == LAG:
                            for h in range(2):
                                ps_o[h] = opool.tile(
                                    [DH + 1, 512], F32, tag="ps_o",
                                    name=f"ps_o_{qb}_{h}")
                        if t >= LAG:
                            emit_attnv(t - LAG, first=(t == LAG))
                        if t in ops:
                            ops[t]()
                    for t in range(KT - LAG, KT):
                        emit_attnv(t, first=False)
                    an = anpool.tile([128, 512], BF16, tag="an",
                                     name=f"an_{qb}")
                    qb_tail(qb, ps_o, an)
                    ops = make_ops(qb, an)
                for t in sorted(ops):
                    ops[t]()

    nc.compile()
    return nc


def _get_nc():
    global _NC_CACHE
    if _NC_CACHE is None:
        _NC_CACHE = _build_nc()
    return _NC_CACHE


def kernel(x, Wq, bq, Wk, bk, Wv, bv, Wo, bo, mask):
    global LAST_EXEC_TIME_NS, LAST_RESULT
    x = np.asarray(x, dtype=np.float32).reshape(S, D)
    Wq = np.asarray(Wq, dtype=np.float32)
    Wk = np.asarray(Wk, dtype=np.float32)
    Wv = np.asarray(Wv, dtype=np.float32)
    Wo = np.asarray(Wo, dtype=np.float32)
    bq = np.asarray(bq, dtype=np.float32)
    bk = np.asarray(bk, dtype=np.float32)
    bv = np.asarray(bv, dtype=np.float32)
    bo = np.asarray(bo, dtype=np.float32)
    mask = np.asarray(mask, dtype=np.float32)

    scale = DH ** -0.5
    xTb = np.ascontiguousarray(x.T.astype(ml_dtypes.bfloat16))    # [D, S]
    maskT_m = np.ascontiguousarray(
        (mask == 0).T.astype(ml_dtypes.bfloat16))                 # [k, q] 1/0
    ident = np.eye(128, dtype=ml_dtypes.bfloat16)

    in_maps = []
    for c in range(NCORES):
        sl = slice(c * F, (c + 1) * F)
        in_maps.append({
            "xT": xTb,
            "WqT": np.ascontiguousarray(
                (Wq[sl, :] * scale).T.astype(ml_dtypes.bfloat16)),
            "WkT": np.ascontiguousarray(Wk[sl, :].T.astype(ml_dtypes.bfloat16)),
            "WvT": np.ascontiguousarray(Wv[sl, :].T.astype(ml_dtypes.bfloat16)),
            "bqv": np.ascontiguousarray((bq[sl] * scale).reshape(F, 1)),
            "bkv": np.ascontiguousarray(bk[sl].reshape(F, 1)),
            "WoT": np.ascontiguousarray(Wo[:, sl].T.astype(ml_dtypes.bfloat16)),
            "maskT": maskT_m,
            "ident": ident,
        })

    nc = _get_nc()
    res = bass_utils.run_bass_kernel_spmd(
        nc, in_maps, core_ids=list(range(NCORES)))
    LAST_EXEC_TIME_NS = res.exec_time_ns
    LAST_RESULT = res

    acc = np.zeros((S, D), dtype=np.float32)
    for c in range(NCORES):
        acc += res.results[c]["out"].astype(np.float32)
    acc += bv @ Wo.T + bo
    return acc.reshape(1, S, D)


# revision 13
# speedup vs baseline: 1.0032x; 1.0032x over previous
"""BigBird attention (S=4096, D=1024, H=16, window=256, 1 global, 32 random)
on 8 TRN2 NeuronCores, head-sharded (2 heads / core).

v2: qb-pair t-major attention. Per core c (heads 2c..2c+1, features c*128..):
  - phase B: projections kT/vT (+qT for qb0/qb1), sc-chunk-paired matmuls to
    amortize stationary reloads
  - phase C: 4 query-block pairs (1024 queries each), t-major over 32 ktiles:
    scores (row-tiled 2-head concurrent, stationary kept resident across the
    pair), exp on ScalarE (the 1 elem/cycle/lane wall), merged mask multiply
    via stride-0 broadcast AP, attnV with ones-column denominators (M=65)
    with stationary reused across the pair
  - normalization + out-proj in a compact pair tail; host sums 8 partials
    and adds the constant bv @ Wo.T + bo term.
"""

import os
from contextlib import ExitStack

import numpy as np
import ml_dtypes

import concourse.bass as bass
import concourse.bacc as bacc
import concourse.tile as tile
from concourse import mybir
import concourse.bass_utils as bass_utils

bass_utils.upload_artifacts = lambda tmpdir: "local"

F32 = mybir.dt.float32
BF16 = mybir.dt.bfloat16
AFT = mybir.ActivationFunctionType

S = 4096          # sequence length
D = 1024          # d_model
F = 128           # features per core (2 heads x 64)
DH = 64           # head dim
NCORES = 8
KF = 8            # contraction chunks of 128 over D
SC = 8            # seq chunks of 512 (projections)
KT = 32           # key tiles of 128
QB = 8            # query blocks of 512
VW = 2 * (DH + 1)  # per-ktile v_aug width (65 per head)

LAST_EXEC_TIME_NS = None
LAST_RESULT = None

_NC_CACHE = None


def _build_nc():
    nc = bacc.Bacc("TRN2", target_bir_lowering=False, debug=False)

    xT = nc.declare_dram_parameter("xT", [D, S], BF16, isOutput=False)
    WqT = nc.declare_dram_parameter("WqT", [D, F], BF16, isOutput=False)
    WkT = nc.declare_dram_parameter("WkT", [D, F], BF16, isOutput=False)
    WvT = nc.declare_dram_parameter("WvT", [D, F], BF16, isOutput=False)
    bqv = nc.declare_dram_parameter("bqv", [F, 1], F32, isOutput=False)
    bkv = nc.declare_dram_parameter("bkv", [F, 1], F32, isOutput=False)
    WoT = nc.declare_dram_parameter("WoT", [F, D], BF16, isOutput=False)
    maskT = nc.declare_dram_parameter("maskT", [S, S], BF16, isOutput=False)
    ident = nc.declare_dram_parameter("ident", [128, 128], BF16, isOutput=False)
    out = nc.declare_dram_parameter("out", [S, D], BF16, isOutput=True)

    with tile.TileContext(nc) as tc:
        with ExitStack() as ctx:
            # ---- persistent sbuf ----
            wpool = ctx.enter_context(tc.tile_pool(name="w", bufs=1))
            wq = wpool.tile([128, D], BF16, tag="wq")
            wk = wpool.tile([128, D], BF16, tag="wk")
            wv = wpool.tile([128, D], BF16, tag="wv")
            for kf in range(KF):
                sl = slice(kf * 128, (kf + 1) * 128)
                nc.sync.dma_start(wq[:, sl], WqT[sl, :])
                nc.sync.dma_start(wk[:, sl], WkT[sl, :])
                nc.sync.dma_start(wv[:, sl], WvT[sl, :])
            wo = wpool.tile([128, D], BF16, tag="wo")
            nc.sync.dma_start(wo[:], WoT[:, :])
            bq_sb = wpool.tile([128, 1], F32, tag="bq")
            bk_sb = wpool.tile([128, 1], F32, tag="bk")
            nc.sync.dma_start(bq_sb[:], bqv[:, :])
            nc.sync.dma_start(bk_sb[:], bkv[:, :])
            id_sb = wpool.tile([128, 128], BF16, tag="id")
            nc.sync.dma_start(id_sb[:], ident[:, :])

            qTs = [wpool.tile([128, 512], BF16, tag=f"qT{i}", name=f"qT{i}")
                   for i in range(QB)]
            kTs = [wpool.tile([128, 512], BF16, tag=f"kT{i}", name=f"kT{i}")
                   for i in range(SC)]
            vs = [wpool.tile([128, VW], BF16, tag=f"v{t}", name=f"v{t}")
                  for t in range(KT)]
            for t in range(KT):
                nc.vector.memset(vs[t][:], 1.0)  # ones cols survive at 64/129

            def kt_ap(h, t):  # [64, 128] head h, key tile t
                return kTs[t // 4][h * DH:(h + 1) * DH,
                                   (t % 4) * 128:(t % 4 + 1) * 128]

            def qt_ap(h, qb):  # [64, 512] head h, query block qb
                return qTs[qb][h * DH:(h + 1) * DH, :]

            xpool = ctx.enter_context(tc.tile_pool(name="xs", bufs=24))

            # ---- phase B: projections, sc-pair stationary reuse ----
            with (
                tc.tile_pool(name="vt", bufs=2) as vtpool,
                tc.tile_pool(name="prps", bufs=4, space="PSUM") as prpool,
                tc.tile_pool(name="qps", bufs=2, space="PSUM") as qppool,
                tc.tile_pool(name="tpps", bufs=2, space="PSUM") as tppool,
            ):
                xts = {}

                def load_x(sc):
                    for kf in range(KF):
                        ksl = slice(kf * 128, (kf + 1) * 128)
                        ssl = slice(sc * 512, (sc + 1) * 512)
                        t_ = xpool.tile([128, 512], BF16, tag="xt",
                                        name=f"xt_{sc}_{kf}")
                        nc.gpsimd.dma_start(t_[:], xT[ksl, ssl])
                        xts[(sc, kf)] = t_

                for sc in range(2):
                    load_x(sc)

                for scp in range(4):
                    scA, scB = 2 * scp, 2 * scp + 1
                    if scA + 2 < SC:
                        load_x(scA + 2)
                    if scB + 2 < SC:
                        load_x(scB + 2)
                    pskA = prpool.tile([128, 512], F32, tag="pr",
                                        name=f"pskA_{scp}")
                    psvA = prpool.tile([128, 512], F32, tag="pr",
                                        name=f"psvA_{scp}")
                    pskB = prpool.tile([128, 512], F32, tag="pr",
                                        name=f"pskB_{scp}")
                    psvB = prpool.tile([128, 512], F32, tag="pr",
                                        name=f"psvB_{scp}")
                    for kf in range(KF):
                        ksl = slice(kf * 128, (kf + 1) * 128)
                        st, sp = kf == 0, kf == KF - 1
                        xA, xB = xts[(scA, kf)][:], xts[(scB, kf)][:]
                        nc.tensor.matmul(pskA[:], wk[:, ksl], xA,
                                         start=st, stop=sp)
                        nc.tensor.matmul(pskB[:], wk[:, ksl], xB,
                                         start=st, stop=sp)
                        nc.tensor.matmul(psvA[:], wv[:, ksl], xA,
                                         start=st, stop=sp)
                        nc.tensor.matmul(psvB[:], wv[:, ksl], xB,
                                         start=st, stop=sp)
                    for sc, psk, psv in ((scA, pskA, psvA), (scB, pskB, psvB)):
                        nc.scalar.activation(kTs[sc][:], psk[:], AFT.Identity,
                                             bias=bk_sb[:], scale=1.0)
                        vt = vtpool.tile([128, 512], BF16)
                        nc.scalar.activation(vt[:], psv[:], AFT.Identity)
                        for j in range(4):
                            t_g = sc * 4 + j
                            pst = tppool.tile([128, 128], BF16, tag="pst")
                            nc.tensor.transpose(pst[:],
                                                vt[:, j * 128:(j + 1) * 128],
                                                id_sb[:])
                            dst = vs[t_g][:].rearrange("p (h x) -> p h x",
                                                       h=2)[:, :, 0:DH]
                            src = pst[:].rearrange("p (h x) -> p h x", h=2)
                            nc.vector.tensor_copy(dst, src)

                    if scp == 0:
                        # q projections for qb0/qb1 while their x tiles live
                        for qb in (0, 1):
                            psq = qppool.tile([128, 512], F32, tag="psq",
                                              name=f"psqB_{qb}")
                            for kf in range(KF):
                                ksl = slice(kf * 128, (kf + 1) * 128)
                                nc.tensor.matmul(psq[:], wq[:, ksl],
                                                 xts[(qb, kf)][:],
                                                 start=(kf == 0),
                                                 stop=(kf == KF - 1))
                            nc.scalar.activation(qTs[qb][:], psq[:],
                                                 AFT.Identity,
                                                 bias=bq_sb[:], scale=1.0)


            # ---- phase C: attention, single-qb t-loops ----
            # PSUM: spool 2x[128,1024] (4 banks) + opool 3x[*,512] (3) +
            # popool 1 (dedicated out-proj bank -> no ps_s ring contention)
            LAG = 2
            with (
                tc.tile_pool(name="mask", bufs=10) as mpool,
                tc.tile_pool(name="attn", bufs=10) as atpool,
                tc.tile_pool(name="an", bufs=2) as anpool,
                tc.tile_pool(name="dn", bufs=8) as dpool,
                tc.tile_pool(name="rc", bufs=4) as rpool,
                tc.tile_pool(name="og", bufs=3) as ogpool,
                tc.tile_pool(name="sps", bufs=2, space="PSUM") as spool,
                tc.tile_pool(name="ops", bufs=3, space="PSUM") as opool,
                tc.tile_pool(name="pps", bufs=1, space="PSUM") as popool,
            ):
                def make_ops(qb, an):
                    # out-proj pieces, scattered into the next qb's t-loop;
                    # po has a dedicated bank so scores never wait on it
                    def op_piece(stt):
                        def go():
                            for oc in range(2):
                                po = popool.tile([128, 512], F32, tag="po",
                                                 name=f"po_{qb}_{stt}_{oc}")
                                nc.tensor.matmul(
                                    po[:],
                                    an[:, stt * 128:(stt + 1) * 128],
                                    wo[:, oc * 512:(oc + 1) * 512],
                                    start=True, stop=True)
                                og = ogpool.tile([128, 512], BF16, tag="og",
                                                 name=f"og_{qb}_{stt}_{oc}")
                                nc.vector.tensor_copy(og[:], po[:])
                                r0 = qb * 512 + stt * 128
                                nc.sync.dma_start(
                                    out[r0:r0 + 128,
                                        oc * 512:(oc + 1) * 512], og[:])
                        return go

                    return {2: op_piece(0), 6: op_piece(1),
                            10: op_piece(2), 14: op_piece(3)}

                def qb_tail(qb, ps_o, an):
                    # q projection for qb+2 FIRST (x prefetched at loop
                    # start, psq borrows a free ps_s slot) so the PE FIFO
                    # never waits on the V-engine epilogues below
                    if qb + 2 < QB:
                        qb_n = qb + 2
                        psq = spool.tile([128, 512], F32, tag="ps_s",
                                         name=f"psq_{qb_n}")
                        for kf in range(KF):
                            ksl2 = slice(kf * 128, (kf + 1) * 128)
                            nc.tensor.matmul(psq[:], wq[:, ksl2],
                                             xqs[(qb_n, kf)][:],
                                             start=(kf == 0),
                                             stop=(kf == KF - 1))
                        nc.scalar.activation(qTs[qb_n][:], psq[:],
                                             AFT.Identity,
                                             bias=bq_sb[:], scale=1.0)
                    # normalize at qb end
                    for h in range(2):
                        po_ = ps_o[h]
                        dn = dpool.tile([1, 512], F32, tag="dn",
                                        name=f"dn_{qb}_{h}")
                        nc.vector.tensor_copy(dn[:], po_[DH:DH + 1, :])
                        rc1 = dpool.tile([1, 512], F32, tag="rc1",
                                         name=f"rc1_{qb}_{h}")
                        nc.vector.reciprocal_approx_fast(rc1[:], dn[:])
                        rcb = rpool.tile([DH, 512], F32, tag="rc",
                                         name=f"rc_{qb}_{h}")
                        nc.gpsimd.partition_broadcast(rcb[:], rc1[:])
                        nc.vector.tensor_mul(an[h * DH:(h + 1) * DH, :],
                                             po_[0:DH, :], rcb[:])

                ops = {}
                xqs = {}
                for qb in range(QB):
                    qsl = slice(qb * 512, (qb + 1) * 512)
                    ps_o = {}
                    ats = {}

                    def emit_attnv(t, first):
                        at = ats.pop(t)
                        for h in range(2):
                            vst = vs[t][:, h * (DH + 1):(h + 1) * (DH + 1)]
                            nc.tensor.matmul(
                                ps_o[h][:], vst,
                                at[:, h * 512:(h + 1) * 512],
                                start=first, stop=(t == KT - 1))

                    for t in range(KT):
                        tsl = slice(t * 128, (t + 1) * 128)
                        msk = mpool.tile([128, 512], BF16)
                        nc.sync.dma_start(msk[:], maskT[tsl, qsl])
                        if qb + 2 < QB and t == 0:
                            # prefetch x for qb+2's q projection
                            qb_n = qb + 2
                            ssl2 = slice(qb_n * 512, (qb_n + 1) * 512)
                            for kf in range(KF):
                                ksl2 = slice(kf * 128, (kf + 1) * 128)
                                xt2 = xpool.tile([128, 512], BF16, tag="xt",
                                                 name=f"xt2_{qb_n}_{kf}")
                                nc.gpsimd.dma_start(xt2[:], xT[ksl2, ssl2])
                                xqs[(qb_n, kf)] = xt2
                        ps_s = spool.tile([128, 1024], F32, tag="ps_s",
                                          name=f"ps_s_{qb}_{t}")
                        for h in range(2):
                            nc.tensor.matmul(
                                ps_s[:, h * 512:(h + 1) * 512],
                                kt_ap(h, t), qt_ap(h, qb),
                                start=True, stop=True,
                                tile_position=(h * DH, 0))
                        at = atpool.tile([128, 1024], BF16, tag="at",
                                         name=f"at_{qb}_{t}")
                        nc.scalar.activation(at[:], ps_s[:], AFT.Exp)
                        # merged mask multiply via stride-0 broadcast
                        mB = msk[:].unsqueeze(1).broadcast_to([128, 2, 512])
                        at3 = at[:].rearrange("p (o x) -> p o x", o=2)
                        nc.vector.tensor_mul(at3, at3, mB)
                        ats[t] = at
                        if t == LAG:
                            for h in range(2):
                                ps_o[h] = opool.tile(
                                    [DH + 1, 512], F32, tag="ps_o",
                                    name=f"ps_o_{qb}_{h}")
                        if t >= LAG:
                            emit_attnv(t - LAG, first=(t == LAG))
                        if t in ops:
                            ops[t]()
                    for t in range(KT - LAG, KT):
                        emit_attnv(t, first=False)
                    an = anpool.tile([128, 512], BF16, tag="an",
                                     name=f"an_{qb}")
                    qb_tail(qb, ps_o, an)
                    ops = make_ops(qb, an)
                for t in sorted(ops):
                    ops[t]()

    nc.compile()
    return nc


def _get_nc():
    global _NC_CACHE
    if _NC_CACHE is None:
        _NC_CACHE = _build_nc()
    return _NC_CACHE


def kernel(x, Wq, bq, Wk, bk, Wv, bv, Wo, bo, mask):
    global LAST_EXEC_TIME_NS, LAST_RESULT
    x = np.asarray(x, dtype=np.float32).reshape(S, D)
    Wq = np.asarray(Wq, dtype=np.float32)
    Wk = np.asarray(Wk, dtype=np.float32)
    Wv = np.asarray(Wv, dtype=np.float32)
    Wo = np.asarray(Wo, dtype=np.float32)
    bq = np.asarray(bq, dtype=np.float32)
    bk = np.asarray(bk, dtype=np.float32)
    bv = np.asarray(bv, dtype=np.float32)
    bo = np.asarray(bo, dtype=np.float32)
    mask = np.asarray(mask, dtype=np.float32)

    scale = DH ** -0.5
    xTb = np.ascontiguousarray(x.T.astype(ml_dtypes.bfloat16))    # [D, S]
    maskT_m = np.ascontiguousarray(
        (mask == 0).T.astype(ml_dtypes.bfloat16))                 # [k, q] 1/0
    ident = np.eye(128, dtype=ml_dtypes.bfloat16)

    in_maps = []
    for c in range(NCORES):
        sl = slice(c * F, (c + 1) * F)
        in_maps.append({
            "xT": xTb,
            "WqT": np.ascontiguousarray(
                (Wq[sl, :] * scale).T.astype(ml_dtypes.bfloat16)),
            "WkT": np.ascontiguousarray(Wk[sl, :].T.astype(ml_dtypes.bfloat16)),
            "WvT": np.ascontiguousarray(Wv[sl, :].T.astype(ml_dtypes.bfloat16)),
            "bqv": np.ascontiguousarray((bq[sl] * scale).reshape(F, 1)),
            "bkv": np.ascontiguousarray(bk[sl].reshape(F, 1)),
            "WoT": np.ascontiguousarray(Wo[:, sl].T.astype(ml_dtypes.bfloat16)),
            "maskT": maskT_m,
            "ident": ident,
        })

    nc = _get_nc()
    res = bass_utils.run_bass_kernel_spmd(
        nc, in_maps, core_ids=list(range(NCORES)))
    LAST_EXEC_TIME_NS = res.exec_time_ns
    LAST_RESULT = res

    acc = np.zeros((S, D), dtype=np.float32)
    for c in range(NCORES):
        acc += res.results[c]["out"].astype(np.float32)
    acc += bv @ Wo.T + bo
    return acc.reshape(1, S, D)
